# revision 15
# baseline (speedup 1.0000x reference)
"""Trainium2 Bass kernel for nn_CrossAttention (B=4, H=8, D=64, C=512, N=M=2048).

Sharding: 8 cores = batch (4) x head-group (2). Core c handles batch b=c//2
and heads hg*4..hg*4+4 with hg=c%2 (tensor parallel on inner_dim). Each core
emits a full-shape partial y; the host unshard sums the two partials per
batch and adds the output bias (bias on host keeps it exact and frees DVE).

V3 design (from the 187us V2 baseline; PE 148us + ACT-exp 140us both ~75%):
  * pv matmuls run fp8e4 with MatmulPerfMode.DoubleRow: two j-chunks
    contracted per instruction at 0.5 cyc/row -> pv 69us -> ~21us PE.
    vaug (v.T, ones col) is stored fp8e4 padded to 80B/chunk (the dual-fp8
    ldweights requires a 16B-aligned k-tile stride).
  * p = exp(sim/8) is written STRAIGHT to fp8e4, into a persistent
    [parity, head, j, 512] SBUF buffer the DR-pv reads through strided APs.
  * the 140us exp stream is split across three engines per j-chunk:
    ACT (exact exp, fp8 out), DVE and Pool (Schraudolph: i8 = round(A*s+B)
    bitcast as fp8e4m3 ~= exp(s/8), std ~3.2%).
  * softmax denominators ride the pv matmul (ones column 64). Normalization:
    one reciprocal of psum row 64 -> bf16, a K=1 ones-matmul broadcasts it
    across partitions into psum, two DVE muls write the normalized out.
    (replaces the raw copies + den DMA + gpsimd partition_broadcast chain)
  * y projection is 4 accumulating K=64 matmuls (one per local head), so
    attention outputs stay on partitions 0:64 and no partition-shift DMA
    is needed.  K<=64 matmuls stream ~2x on this silicon (168ns vs 301ns
    for 512 cols, measured).
"""

from collections import deque
from contextlib import ExitStack

import numpy as np

import concourse.bass as bass
import concourse.mybir as mybir
import concourse.tile as tile
from concourse import bacc
from concourse.bass_utils import run_bass_kernel_spmd

FP = mybir.dt.float32
BF16 = mybir.dt.bfloat16
F8 = mybir.dt.float8e4
I8 = mybir.dt.int8
EXP = mybir.ActivationFunctionType.Exp
DR = mybir.MatmulPerfMode.DoubleRow
F32R = mybir.dt.float32r

P = 128
H, D = 8, 64
C = 512             # query_dim == full inner_dim
N, M = 2048, 2048
HL = 4              # local heads per core
HPL = 2             # local head pairs
CIN = HL * D        # local inner dim = 256
CC = C // P         # 4 contraction chunks for q/k/v projections
IT = N // 512       # 4 query i-tiles
JC = M // P         # 16 context chunks
GC = JC // 2        # 8 j-chunk pairs (one DR pv matmul each)
NT = M // 512       # 4 context column blocks
VPAD = 80           # fp8 vaug stride per (j,h): 65 used, 16B-aligned
ICY = CIN // P      # 2 inner chunks for the y projection
SCALE = float(D) ** -0.5
N_CORES = 8
NWARM = 6

# exp(SCALE*s) ~= bitcast_fp8e4m3(i8 = round(A8*s + B8))  [Schraudolph]
LOG2E = 1.4426950408889634
A8 = 8.0 * LOG2E * SCALE
B8 = 56.0 - 0.42

# per-pass j-chunk -> exp engine (GPSIMD cannot read PSUM, so no Pool
# exp). 10 ACT-exact / 6 DVE-Schraudolph balances ACT against DVE's
# norm + cast load.
DVE_JS = (4, 6, 8, 10, 12, 14)
POOL_JS = ()


DEBUG = False


def _build_program():
    nc = bacc.Bacc("TRN2")
    x = nc.dram_tensor("x", [P, IT * CC * 512], BF16, kind="ExternalInput")
    ctx = nc.dram_tensor("ctx", [P, NT * CC * 512], BF16, kind="ExternalInput")
    wq = nc.dram_tensor("wq", [P, CC * CIN], BF16, kind="ExternalInput")
    wk = nc.dram_tensor("wk", [P, CC * CIN], BF16, kind="ExternalInput")
    wv = nc.dram_tensor("wv", [P, CC * CIN], BF16, kind="ExternalInput")
    wo = nc.dram_tensor("wo", [P, (CIN // P) * 512], BF16, kind="ExternalInput")
    y = nc.dram_tensor("y", [P, CC * N], BF16, kind="ExternalOutput")
    dbg = None
    if DEBUG:
        dbg = {
            "dbg_q": nc.dram_tensor("dbg_q", [P, HPL * N], BF16, kind="ExternalOutput"),
            "dbg_k": nc.dram_tensor("dbg_k", [P, HPL * M], BF16, kind="ExternalOutput"),
            "dbg_vaug": nc.dram_tensor("dbg_vaug", [P, JC * HL * VPAD], F8, kind="ExternalOutput"),
            "dbg_p": nc.dram_tensor("dbg_p", [P, 2 * 2 * JC * 512], F8, kind="ExternalOutput"),
            "dbg_out4": nc.dram_tensor("dbg_out4", [P, ICY * N], BF16, kind="ExternalOutput"),
            "dbg_pv": nc.dram_tensor("dbg_pv", [65, 1024], FP, kind="ExternalOutput"),
            "dbg_rec": nc.dram_tensor("dbg_rec", [1, 1024], FP, kind="ExternalOutput"),
            "dbg_den": nc.dram_tensor("dbg_den", [1, 1024], FP, kind="ExternalOutput"),
            "dbg_bc": nc.dram_tensor("dbg_bc", [P, 1024], FP, kind="ExternalOutput"),
        }

    with tile.TileContext(nc) as tc:
        _emit(tc, x, ctx, wq, wk, wv, wo, y, dbg)
    nc.finalize()
    return nc


def _emit(tc, x, ctx, wq, wk, wv, wo, y, dbg=None):
    nc = tc.nc
    with ExitStack() as st:
        wpool = st.enter_context(tc.tile_pool(name="weights", bufs=1))
        apool = st.enter_context(tc.tile_pool(name="acts", bufs=1))
        spool = st.enter_context(tc.tile_pool(name="small", bufs=2))
        ypool = st.enter_context(tc.tile_pool(name="ystage", bufs=4))
        psim = st.enter_context(tc.tile_pool(name="psim", bufs=2, space="PSUM"))
        ppv = st.enter_context(tc.tile_pool(name="ppv", bufs=1, space="PSUM"))
        pmisc = st.enter_context(tc.tile_pool(name="pmisc", bufs=2, space="PSUM"))

        # ---- input loads: ONE queue, strict priority order ----
        wq_s = wpool.tile([P, CC * CIN], BF16, tag="wq")
        nc.sync.dma_start(out=wq_s, in_=wq[:, :])
        x_s = apool.tile([P, IT * CC * 512], BF16, tag="x")
        nc.sync.dma_start(out=x_s[:, 0:2048], in_=x[:, 0:2048])
        wk_s = wpool.tile([P, CC * CIN], BF16, tag="wk")
        nc.sync.dma_start(out=wk_s, in_=wk[:, :])
        ctx_s = apool.tile([P, NT * CC * 512], BF16, tag="ctx")
        nc.sync.dma_start(out=ctx_s[:, 0:2048], in_=ctx[:, 0:2048])
        wv_s = wpool.tile([P, CC * CIN], BF16, tag="wv")
        nc.sync.dma_start(out=wv_s, in_=wv[:, :])
        for b in range(1, NT):
            nc.sync.dma_start(
                out=ctx_s[:, b * 2048:(b + 1) * 2048],
                in_=ctx[:, b * 2048:(b + 1) * 2048],
            )
            nc.sync.dma_start(
                out=x_s[:, b * 2048:(b + 1) * 2048],
                in_=x[:, b * 2048:(b + 1) * 2048],
            )
        wo_s = wpool.tile([P, ICY * 512], BF16, tag="wo")
        nc.sync.dma_start(out=wo_s, in_=wo[:, :])

        # ---- persistent SBUF intermediates ----
        q_s = apool.tile([P, HPL * N], BF16, tag="q")
        k_s = apool.tile([P, HPL * M], BF16, tag="k")
        # vaug fp8: j-chunk j, local head h at cols (j*HL + h)*VPAD; col 64=1
        vaug = apool.tile([P, JC * HL * VPAD], F8, tag="vaug")
        vaug4 = vaug.rearrange("p (j h e) -> p j h e", j=JC, h=HL)
        # attention out: [128 parts = 2 heads x 64 d] ic chunk at ic*N+it*512
        out_s = apool.tile([P, ICY * N], BF16, tag="out")
        # p fp8: [parity, head-half, j, 512]
        p_s = apool.tile([P, 2 * 2 * JC * 512], F8, tag="p")
        p_v = p_s.rearrange("p (par hh j n) -> p par hh j n", par=2, hh=2, j=JC)

        ones_s = wpool.tile([P, P], BF16, tag="ones")
        nc.vector.memset(ones_s, 1.0)
        ones4 = ones_s[:, 0:JC * HL].rearrange("p (j h e) -> p j h e", j=JC, h=HL)
        nc.vector.tensor_copy(out=vaug4[:, :, :, 64:65], in_=ones4)

        # HAM warmup: burn matmuls during the initial DMA wait so the first
        # projections run at full clock.
        warm = pmisc.tile([P, 512], FP, tag="scratch", name="warm")
        for w in range(NWARM):
            nc.tensor.matmul(warm[:, 0:P], lhsT=ones_s[:, 0:P],
                             rhs=ones_s[:, 0:P],
                             start=(w == 0), stop=(w == NWARM - 1))
        warm_sink = spool.tile([P, P], FP, tag="warmsink", bufs=1)
        nc.vector.tensor_copy(out=warm_sink, in_=warm[:, 0:P])

        def proj_qk(dst, w_s, oc, rhs_of_cc):
            """One [128, 512] q/k projection tile (local head pair oc).
            PSUM->SBUF cast on DVE (GPSIMD cannot access PSUM)."""
            pt = pmisc.tile([P, 512], FP, tag="scratch")
            for cc in range(CC):
                nc.tensor.matmul(
                    pt,
                    lhsT=w_s[:, cc * CIN + oc * P: cc * CIN + (oc + 1) * P],
                    rhs=rhs_of_cc(cc),
                    start=(cc == 0), stop=(cc == CC - 1),
                )
            nc.vector.tensor_copy(out=dst, in_=pt)

        def emit_q(oc, it):
            proj_qk(q_s[:, oc * N + it * 512: oc * N + (it + 1) * 512], wq_s, oc,
                    lambda cc: x_s[:, it * 2048 + cc * 512: it * 2048 + (cc + 1) * 512])

        def emit_k(oc, nt):
            proj_qk(k_s[:, oc * M + nt * 512: oc * M + (nt + 1) * 512], wk_s, oc,
                    lambda cc: ctx_s[:, nt * 2048 + cc * 512: nt * 2048 + (cc + 1) * 512])

        def emit_v(j):
            nb, jm = j // 4, j % 4
            pt = pmisc.tile([P, 512], FP, tag="scratch")
            for cc in range(CC):
                nc.tensor.matmul(
                    pt[:, 0:CIN],
                    lhsT=ctx_s[:, nb * 2048 + cc * 512 + jm * P:
                               nb * 2048 + cc * 512 + (jm + 1) * P],
                    rhs=wv_s[:, cc * CIN:(cc + 1) * CIN],
                    start=(cc == 0), stop=(cc == CC - 1),
                )
            nc.vector.tensor_copy(
                out=vaug4[:, j, :, 0:64],
                in_=pt[:, 0:CIN].rearrange("p (h e) -> p h e", h=HL),
            )

        def emit_y(oc, nt2):
            """y chunk: ICY accumulating K=128 matmuls."""
            pt = pmisc.tile([P, 512], FP, tag="scratch")
            for ic in range(ICY):
                nc.tensor.matmul(
                    pt,
                    lhsT=wo_s[:, ic * 512 + oc * P: ic * 512 + (oc + 1) * P],
                    rhs=out_s[:, ic * N + nt2 * 512: ic * N + (nt2 + 1) * 512],
                    start=(ic == 0), stop=(ic == ICY - 1),
                )
            ys = ypool.tile([P, 512], BF16, tag="ys")
            nc.vector.tensor_copy(out=ys, in_=pt)
            nc.sync.dma_start(
                out=y[:, oc * N + nt2 * 512: oc * N + (nt2 + 1) * 512], in_=ys)

        # pinned[i]: projection tiles that MUST be emitted during pass i;
        # free: y chunks drained opportunistically.
        pinned = {i: deque() for i in range(HPL * IT)}
        # pass 0 sims j=4t need k(0,t) already emitted: k(0,1) goes before
        # pass_main (see loop below); k(0,2)/k(0,3) land at slots 0/2,
        # which precede sim(8)/sim(12).
        pinned[0].append(lambda: emit_k(0, 2))
        pinned[0].append(lambda: emit_q(0, 1))
        pinned[0].append(lambda: emit_k(0, 3))
        pinned[1].append(lambda: emit_q(0, 2))
        pinned[1].append(lambda: emit_k(1, 0))
        pinned[1].append(lambda: emit_k(1, 1))
        pinned[2].append(lambda: emit_q(0, 3))
        pinned[2].append(lambda: emit_k(1, 2))
        pinned[2].append(lambda: emit_k(1, 3))
        pinned[2].append(lambda: emit_q(1, 0))
        for it in range(1, IT):
            pinned[2 + it].append(lambda it=it: emit_q(1, it))
        free = deque()

        # upfront: only what pass (0,0)'s first js need
        emit_q(0, 0)
        emit_k(0, 0)

        def pass_begin(hp, it, emit_v_inline):
            c = {}
            c["hp"], c["it"], c["v_inline"] = hp, it, emit_v_inline
            c["par"] = (hp * IT + it) % 2
            c["pv"] = ppv.tile([65, 1024], FP, tag="pv", name="pvb")
            c["qA"] = q_s[0:64, hp * N + it * 512: hp * N + (it + 1) * 512]
            c["qB"] = q_s[64:128, hp * N + it * 512: hp * N + (it + 1) * 512]
            c["jtiles"] = [None] * JC
            emit_sim(c, 0)
            emit_exp(c, 0)
            return c

        def emit_sim(c, j):
            hp = c["hp"]
            stt = psim.tile([P, 1024], FP, tag="sim", name="st_t")
            c["jtiles"][j] = stt
            for half in range(2):
                nc.tensor.matmul(
                    stt[:, half * 512:(half + 1) * 512],
                    lhsT=k_s[half * 64:(half + 1) * 64,
                             hp * M + j * P: hp * M + (j + 1) * P],
                    rhs=(c["qA"] if half == 0 else c["qB"]),
                )
            if c["v_inline"]:
                emit_v(j)

        def emit_exp(c, j):
            stt = c["jtiles"][j]
            in_v = stt.rearrange("p (hh n) -> p hh n", hh=2)
            out_v = p_v[:, c["par"], :, j, :]
            if j in DVE_JS:
                nc.vector.tensor_scalar(
                    out=out_v.bitcast(I8), in0=in_v,
                    scalar1=A8, scalar2=B8,
                    op0=mybir.AluOpType.mult, op1=mybir.AluOpType.add)
            elif j in POOL_JS:
                nc.gpsimd.tensor_scalar(
                    out=out_v.bitcast(I8), in0=in_v,
                    scalar1=A8, scalar2=B8,
                    op0=mybir.AluOpType.mult, op1=mybir.AluOpType.add)
            else:
                nc.scalar.activation(out=out_v, in_=in_v, func=EXP, scale=SCALE)

        def emit_pv(c, g):
            hp, par = c["hp"], c["par"]
            for half in range(2):
                h = 2 * hp + half
                nc.tensor.matmul(
                    c["pv"][0:65, half * 512:(half + 1) * 512],
                    lhsT=vaug4[:, 2 * g:2 * g + 2, h, 0:65],
                    rhs=p_v[:, par, half, 2 * g:2 * g + 2, :],
                    perf_mode=DR,
                    start=(g == 0), stop=(g == GC - 1),
                )

        def pass_main(c, mine):
            # sims run 4 j-chunks ahead of pv so the previous pass's norm
            # chain (den->rec->broadcast->muls) never blocks pv(0) on the
            # single pv psum buffer.
            for g in range(GC - 2):
                emit_sim(c, 2 * g + 4)
                emit_exp(c, 2 * g + 4)
                emit_sim(c, 2 * g + 5)
                emit_exp(c, 2 * g + 5)
                emit_pv(c, g)
                if mine:
                    mine.popleft()()
                elif free:
                    free.popleft()()

        def pass_finish(c):
            """pv(7) + normalization (emitted after the NEXT pass's first
            sim+exp).  Denominators sit at psum partition 64; reciprocal ->
            bf16, K=1 ones-matmul broadcasts 1/den across partitions into
            psum, two muls write the normalized [64, 512] out tiles."""
            emit_pv(c, GC - 2)
            emit_pv(c, GC - 1)
            pv, hp, it = c["pv"], c["hp"], c["it"]
            den = spool.tile([1, 1024], FP, tag="den", bufs=2)
            nc.scalar.copy(out=den, in_=pv[64:65, :])
            rec = spool.tile([1, 1024], FP, tag="rec", bufs=2)
            nc.vector.reciprocal_approx_fast(out=rec, in_=den)
            if dbg is not None and hp == 0 and it == 0:
                pvd = spool.tile([65, 1024], FP, tag="pvd", bufs=1)
                nc.vector.tensor_copy(out=pvd, in_=pv)
                nc.sync.dma_start(out=dbg["dbg_pv"][:, :], in_=pvd)
                nc.sync.dma_start(out=dbg["dbg_rec"][:, :], in_=rec)
                nc.sync.dma_start(out=dbg["dbg_den"][:, :], in_=den)
            bc = spool.tile([P, 1024], FP, tag="bc", bufs=2)
            nc.gpsimd.partition_broadcast(bc, rec[0:1, :])
            if dbg is not None and hp == 0 and it == 0:
                nc.sync.dma_start(out=dbg["dbg_bc"][:, :], in_=bc)
            ocol = hp * N + it * 512
            nc.vector.tensor_mul(
                out=out_s[0:64, ocol:ocol + 512],
                in0=pv[0:64, 0:512], in1=bc[0:64, 0:512])
            bb = spool.tile([D, 512], BF16, tag="bb", bufs=2)
            nc.vector.tensor_mul(
                out=bb, in0=pv[0:64, 512:1024], in1=bc[0:64, 512:1024])
            nc.sync.dma_start(out=out_s[64:128, ocol:ocol + 512], in_=bb)
            if hp == 1:
                for oc in range(CC):
                    free.append(lambda oc=oc, it=it: emit_y(oc, it))

        prev = None
        for hp in range(HPL):
            for it in range(IT):
                c = pass_begin(hp, it, emit_v_inline=(hp == 0 and it == 0))
                emit_sim(c, 1)
                emit_exp(c, 1)
                if prev is not None:
                    pass_finish(prev)
                emit_sim(c, 2)
                emit_exp(c, 2)
                emit_sim(c, 3)
                emit_exp(c, 3)
                if hp == 0 and it == 0:
                    emit_k(0, 1)
                pass_main(c, pinned[hp * IT + it])
                prev = c
        pass_finish(prev)
        while free:
            free.popleft()()
        if dbg is not None:
            nc.sync.dma_start(out=dbg["dbg_q"][:, :], in_=q_s)
            nc.sync.dma_start(out=dbg["dbg_k"][:, :], in_=k_s)
            nc.sync.dma_start(out=dbg["dbg_vaug"][:, :], in_=vaug)
            nc.sync.dma_start(out=dbg["dbg_p"][:, :], in_=p_s)
            nc.sync.dma_start(out=dbg["dbg_out4"][:, :], in_=out_s)


# ------------------------- host-side shard / gather -------------------------

def _bf16(a):
    import ml_dtypes
    return np.ascontiguousarray(a.astype(ml_dtypes.bfloat16))


def _shard_inputs(x, context, Wq, Wk, Wv, Wo, bo):
    """Build the per-core DRAM images (all [partitions, free])."""
    def chunk_rows(a):
        n = a.shape[1]
        return np.ascontiguousarray(
            a.reshape(-1, P, n).transpose(1, 0, 2).reshape(P, -1))

    WqT, WkT, WvT, WoT = Wq.T, Wk.T, Wv.T, Wo.T

    in_maps = []
    for c in range(N_CORES):
        b, hg = c // 2, c % 2
        cols = slice(hg * CIN, (hg + 1) * CIN)
        x_s = x[b].reshape(CC, P, IT, 512).transpose(1, 2, 0, 3).reshape(P, IT * CC * 512)
        ctx_s = context[b].reshape(CC, P, NT, 512).transpose(1, 2, 0, 3).reshape(P, NT * CC * 512)
        in_maps.append({
            "x": _bf16(x_s),
            "ctx": _bf16(ctx_s),
            "wq": _bf16(chunk_rows(np.ascontiguousarray(WqT[:, cols]))),
            "wk": _bf16(chunk_rows(np.ascontiguousarray(WkT[:, cols]))),
            "wv": _bf16(chunk_rows(np.ascontiguousarray(WvT[:, cols]))),
            "wo": _bf16(chunk_rows(np.ascontiguousarray(WoT[hg * CIN:(hg + 1) * CIN, :]))),
        })
    return in_maps


def _gather_outputs(results, bo):
    y_full = np.empty((4, C, N), np.float32)
    for b in range(4):
        acc = None
        for hg in range(2):
            y_s = np.asarray(results[2 * b + hg]["y"], np.float32)  # [128, 4*2048]
            part = y_s.reshape(P, CC, N).transpose(1, 0, 2).reshape(C, N)
            acc = part if acc is None else acc + part
        y_full[b] = acc + bo[:, None]
    return y_full


_PROGRAM = None


def _get_program():
    global _PROGRAM
    if _PROGRAM is None:
        _PROGRAM = _build_program()
    return _PROGRAM


def run(trace=False, **inputs):
    nc = _get_program()
    bo = np.asarray(inputs["bo"], np.float32)
    in_maps = _shard_inputs(
        np.asarray(inputs["x"], np.float32),
        np.asarray(inputs["context"], np.float32),
        np.asarray(inputs["Wq"], np.float32),
        np.asarray(inputs["Wk"], np.float32),
        np.asarray(inputs["Wv"], np.float32),
        np.asarray(inputs["Wo"], np.float32),
        bo,
    )
    res = run_bass_kernel_spmd(nc, in_maps, list(range(N_CORES)), trace=trace)
    return _gather_outputs(res.results, bo), res


def kernel(**inputs):
    out, _ = run(trace=False, **inputs)
    return out


# revision 16
# speedup vs baseline: 1.0183x; 1.0183x over previous
"""Trainium2 Bass kernel for nn_CrossAttention (B=4, H=8, D=64, C=512, N=M=2048).

Sharding: 8 cores = batch (4) x head-group (2). Core c handles batch b=c//2
and heads hg*4..hg*4+4 with hg=c%2 (tensor parallel on inner_dim). Each core
emits a full-shape partial y; the host unshard sums the two partials per
batch and adds the output bias (bias on host keeps it exact and frees DVE).

V3 design (from the 187us V2 baseline; PE 148us + ACT-exp 140us both ~75%):
  * pv matmuls run fp8e4 with MatmulPerfMode.DoubleRow: two j-chunks
    contracted per instruction at 0.5 cyc/row -> pv 69us -> ~21us PE.
    vaug (v.T, ones col) is stored fp8e4 padded to 80B/chunk (the dual-fp8
    ldweights requires a 16B-aligned k-tile stride).
  * p = exp(sim/8) is written STRAIGHT to fp8e4, into a persistent
    [parity, head, j, 512] SBUF buffer the DR-pv reads through strided APs.
  * the 140us exp stream is split across three engines per j-chunk:
    ACT (exact exp, fp8 out), DVE and Pool (Schraudolph: i8 = round(A*s+B)
    bitcast as fp8e4m3 ~= exp(s/8), std ~3.2%).
  * softmax denominators ride the pv matmul (ones column 64). Normalization:
    one reciprocal of psum row 64 -> bf16, a K=1 ones-matmul broadcasts it
    across partitions into psum, two DVE muls write the normalized out.
    (replaces the raw copies + den DMA + gpsimd partition_broadcast chain)
  * y projection is 4 accumulating K=64 matmuls (one per local head), so
    attention outputs stay on partitions 0:64 and no partition-shift DMA
    is needed.  K<=64 matmuls stream ~2x on this silicon (168ns vs 301ns
    for 512 cols, measured).
"""

from collections import deque
from contextlib import ExitStack

import numpy as np

import concourse.bass as bass
import concourse.mybir as mybir
import concourse.tile as tile
from concourse import bacc
from concourse.bass_utils import run_bass_kernel_spmd

FP = mybir.dt.float32
BF16 = mybir.dt.bfloat16
F8 = mybir.dt.float8e4
I8 = mybir.dt.int8
EXP = mybir.ActivationFunctionType.Exp
DR = mybir.MatmulPerfMode.DoubleRow
F32R = mybir.dt.float32r

P = 128
H, D = 8, 64
C = 512             # query_dim == full inner_dim
N, M = 2048, 2048
HL = 4              # local heads per core
HPL = 2             # local head pairs
CIN = HL * D        # local inner dim = 256
CC = C // P         # 4 contraction chunks for q/k/v projections
IT = N // 512       # 4 query i-tiles
JC = M // P         # 16 context chunks
GC = JC // 2        # 8 j-chunk pairs (one DR pv matmul each)
NT = M // 512       # 4 context column blocks
VPAD = 80           # fp8 vaug stride per (j,h): 65 used, 16B-aligned
ICY = CIN // P      # 2 inner chunks for the y projection
SCALE = float(D) ** -0.5
N_CORES = 8
NWARM = 6

# Wq/Wk/Wv are shipped x16 in fp8 (dodges e4m3 subnormals); q,k are thus
# x16 each and sim x256 -> the exp scale absorbs 1/256 exactly.
WSCALE = 16.0
SIMSCALE = SCALE / (WSCALE * WSCALE)
# exp(SIMSCALE*s) ~= bitcast_fp8e4m3(i8 = round(A8*s + B8))  [Schraudolph]
LOG2E = 1.4426950408889634
A8 = 8.0 * LOG2E * SIMSCALE
B8 = 56.0 - 0.42

# per-pass j-chunk -> exp engine (GPSIMD cannot read PSUM, so no Pool
# exp). 10 ACT-exact / 6 DVE-Schraudolph balances ACT against DVE's
# norm + cast load.
DVE_JS = (4, 6, 8, 10, 12, 14)
POOL_JS = ()


DEBUG = False


def _build_program():
    nc = bacc.Bacc("TRN2")
    x = nc.dram_tensor("x", [P, IT * CC * 512], F8, kind="ExternalInput")
    ctx = nc.dram_tensor("ctx", [P, NT * CC * 512], F8, kind="ExternalInput")
    wq = nc.dram_tensor("wq", [P, CC * CIN], F8, kind="ExternalInput")
    wk = nc.dram_tensor("wk", [P, CC * CIN], F8, kind="ExternalInput")
    wv = nc.dram_tensor("wv", [P, CC * CIN], F8, kind="ExternalInput")
    wo = nc.dram_tensor("wo", [P, (CIN // P) * 512], BF16, kind="ExternalInput")
    y = nc.dram_tensor("y", [P, CC * N], BF16, kind="ExternalOutput")
    dbg = None
    if DEBUG:
        dbg = {
            "dbg_q": nc.dram_tensor("dbg_q", [P, HPL * N], BF16, kind="ExternalOutput"),
            "dbg_k": nc.dram_tensor("dbg_k", [P, HPL * M], BF16, kind="ExternalOutput"),
            "dbg_vaug": nc.dram_tensor("dbg_vaug", [P, JC * HL * VPAD], F8, kind="ExternalOutput"),
            "dbg_p": nc.dram_tensor("dbg_p", [P, 2 * 2 * JC * 512], F8, kind="ExternalOutput"),
            "dbg_out4": nc.dram_tensor("dbg_out4", [P, ICY * N], BF16, kind="ExternalOutput"),
            "dbg_pv": nc.dram_tensor("dbg_pv", [65, 1024], FP, kind="ExternalOutput"),
            "dbg_rec": nc.dram_tensor("dbg_rec", [1, 1024], FP, kind="ExternalOutput"),
            "dbg_den": nc.dram_tensor("dbg_den", [1, 1024], FP, kind="ExternalOutput"),
            "dbg_bc": nc.dram_tensor("dbg_bc", [P, 1024], FP, kind="ExternalOutput"),
        }

    with tile.TileContext(nc) as tc:
        _emit(tc, x, ctx, wq, wk, wv, wo, y, dbg)
    nc.finalize()
    return nc


def _emit(tc, x, ctx, wq, wk, wv, wo, y, dbg=None):
    nc = tc.nc
    with ExitStack() as st:
        wpool = st.enter_context(tc.tile_pool(name="weights", bufs=1))
        apool = st.enter_context(tc.tile_pool(name="acts", bufs=1))
        spool = st.enter_context(tc.tile_pool(name="small", bufs=2))
        ypool = st.enter_context(tc.tile_pool(name="ystage", bufs=4))
        psim = st.enter_context(tc.tile_pool(name="psim", bufs=2, space="PSUM"))
        ppv = st.enter_context(tc.tile_pool(name="ppv", bufs=1, space="PSUM"))
        pmisc = st.enter_context(tc.tile_pool(name="pmisc", bufs=2, space="PSUM"))

        # ---- input loads (all fp8): ONE queue, strict priority order ----
        wq_s = wpool.tile([P, CC * CIN], F8, tag="wq")
        nc.sync.dma_start(out=wq_s, in_=wq[:, :])
        x_s = apool.tile([P, IT * CC * 512], F8, tag="x")
        nc.sync.dma_start(out=x_s[:, 0:2048], in_=x[:, 0:2048])
        wk_s = wpool.tile([P, CC * CIN], F8, tag="wk")
        nc.sync.dma_start(out=wk_s, in_=wk[:, :])
        ctx_s = apool.tile([P, NT * CC * 512], F8, tag="ctx")
        nc.sync.dma_start(out=ctx_s[:, 0:2048], in_=ctx[:, 0:2048])
        wv_s = wpool.tile([P, CC * CIN], F8, tag="wv")
        nc.sync.dma_start(out=wv_s, in_=wv[:, :])
        wq_v = wq_s.rearrange("p (cc m) -> p cc m", cc=CC)
        wk_v = wk_s.rearrange("p (cc m) -> p cc m", cc=CC)
        wv_v = wv_s.rearrange("p (cc m) -> p cc m", cc=CC)
        x_v = x_s.rearrange("p (it cc n) -> p it cc n", it=IT, cc=CC)
        ctx_v = ctx_s.rearrange("p (nt cc n) -> p nt cc n", nt=NT, cc=CC)
        for b in range(1, NT):
            nc.sync.dma_start(
                out=ctx_s[:, b * 2048:(b + 1) * 2048],
                in_=ctx[:, b * 2048:(b + 1) * 2048],
            )
            nc.sync.dma_start(
                out=x_s[:, b * 2048:(b + 1) * 2048],
                in_=x[:, b * 2048:(b + 1) * 2048],
            )
        wo_s = wpool.tile([P, ICY * 512], BF16, tag="wo")
        nc.sync.dma_start(out=wo_s, in_=wo[:, :])

        # ---- persistent SBUF intermediates ----
        q_s = apool.tile([P, HPL * N], BF16, tag="q")
        k_s = apool.tile([P, HPL * M], BF16, tag="k")
        # vaug fp8: j-chunk j, local head h at cols (j*HL + h)*VPAD; col 64=1
        vaug = apool.tile([P, JC * HL * VPAD], F8, tag="vaug")
        vaug4 = vaug.rearrange("p (j h e) -> p j h e", j=JC, h=HL)
        # attention out: [128 parts = 2 heads x 64 d] ic chunk at ic*N+it*512
        out_s = apool.tile([P, ICY * N], BF16, tag="out")
        # p fp8: [parity, head-half, j, 512]
        p_s = apool.tile([P, 2 * 2 * JC * 512], F8, tag="p")
        p_v = p_s.rearrange("p (par hh j n) -> p par hh j n", par=2, hh=2, j=JC)

        ones_s = wpool.tile([P, P], BF16, tag="ones")
        nc.vector.memset(ones_s, 1.0)
        ones4 = ones_s[:, 0:JC * HL].rearrange("p (j h e) -> p j h e", j=JC, h=HL)
        nc.vector.tensor_copy(out=vaug4[:, :, :, 64:65], in_=ones4)

        # HAM warmup: burn matmuls during the initial DMA wait so the first
        # projections run at full clock.
        warm = pmisc.tile([P, 512], FP, tag="scratch", name="warm")
        for w in range(NWARM):
            nc.tensor.matmul(warm[:, 0:P], lhsT=ones_s[:, 0:P],
                             rhs=ones_s[:, 0:P],
                             start=(w == 0), stop=(w == NWARM - 1))
        warm_sink = spool.tile([P, P], FP, tag="warmsink", bufs=1)
        nc.vector.tensor_copy(out=warm_sink, in_=warm[:, 0:P])

        def proj_qk(dst, w_v, oc, act_v):
            """One [128, 512] q/k projection tile via 2 fp8 DoubleRow
            matmuls (cc-chunk pairs); PSUM->SBUF cast on DVE."""
            pt = pmisc.tile([P, 512], FP, tag="scratch")
            for t in range(2):
                nc.tensor.matmul(
                    pt,
                    lhsT=w_v[:, 2 * t:2 * t + 2, oc * P:(oc + 1) * P],
                    rhs=act_v[:, 2 * t:2 * t + 2, :],
                    perf_mode=DR,
                    start=(t == 0), stop=(t == 1),
                )
            nc.vector.tensor_copy(out=dst, in_=pt)

        def emit_q(oc, it):
            proj_qk(q_s[:, oc * N + it * 512: oc * N + (it + 1) * 512],
                    wq_v, oc, x_v[:, it, :, :])

        def emit_k(oc, nt):
            proj_qk(k_s[:, oc * M + nt * 512: oc * M + (nt + 1) * 512],
                    wk_v, oc, ctx_v[:, nt, :, :])

        def emit_v(j):
            nb, jm = j // 4, j % 4
            pt = pmisc.tile([P, 512], FP, tag="scratch")
            for t in range(2):
                nc.tensor.matmul(
                    pt[:, 0:CIN],
                    lhsT=ctx_v[:, nb, 2 * t:2 * t + 2, jm * P:(jm + 1) * P],
                    rhs=wv_v[:, 2 * t:2 * t + 2, :],
                    perf_mode=DR,
                    start=(t == 0), stop=(t == 1),
                )
            # weights were pre-scaled x16 on host; fold 1/16 back here
            nc.vector.tensor_scalar_mul(
                out=vaug4[:, j, :, 0:64],
                in0=pt[:, 0:CIN].rearrange("p (h e) -> p h e", h=HL),
                scalar1=1.0 / 16.0,
            )

        def emit_y(oc, nt2):
            """y chunk: ICY accumulating K=128 matmuls."""
            pt = pmisc.tile([P, 512], FP, tag="scratch")
            for ic in range(ICY):
                nc.tensor.matmul(
                    pt,
                    lhsT=wo_s[:, ic * 512 + oc * P: ic * 512 + (oc + 1) * P],
                    rhs=out_s[:, ic * N + nt2 * 512: ic * N + (nt2 + 1) * 512],
                    start=(ic == 0), stop=(ic == ICY - 1),
                )
            ys = ypool.tile([P, 512], BF16, tag="ys")
            if oc % 2 == 0:
                nc.scalar.copy(out=ys, in_=pt)
            else:
                nc.vector.tensor_copy(out=ys, in_=pt)
            nc.sync.dma_start(
                out=y[:, oc * N + nt2 * 512: oc * N + (nt2 + 1) * 512], in_=ys)

        # pinned[i]: projection tiles that MUST be emitted during pass i;
        # free: y chunks drained opportunistically.
        pinned = {i: deque() for i in range(HPL * IT)}
        # pass 0 sims j=4t need k(0,t) already emitted: k(0,1) goes before
        # pass_main (see loop below); k(0,2)/k(0,3) land at slots 0/2,
        # which precede sim(8)/sim(12).
        pinned[0].append(lambda: emit_k(0, 2))
        pinned[0].append(lambda: emit_q(0, 1))
        pinned[0].append(lambda: emit_k(0, 3))
        pinned[1].append(lambda: emit_q(0, 2))
        pinned[1].append(lambda: emit_k(1, 0))
        pinned[1].append(lambda: emit_k(1, 1))
        pinned[2].append(lambda: emit_q(0, 3))
        pinned[2].append(lambda: emit_k(1, 2))
        pinned[2].append(lambda: emit_k(1, 3))
        pinned[2].append(lambda: emit_q(1, 0))
        for it in range(1, IT):
            pinned[2 + it].append(lambda it=it: emit_q(1, it))
        free = deque()

        # upfront: only what pass (0,0)'s first js need
        emit_q(0, 0)
        emit_k(0, 0)

        def pass_begin(hp, it, emit_v_inline):
            c = {}
            c["hp"], c["it"], c["v_inline"] = hp, it, emit_v_inline
            c["par"] = (hp * IT + it) % 2
            c["pv"] = ppv.tile([65, 1024], FP, tag="pv", name="pvb")
            c["qA"] = q_s[0:64, hp * N + it * 512: hp * N + (it + 1) * 512]
            c["qB"] = q_s[64:128, hp * N + it * 512: hp * N + (it + 1) * 512]
            c["jtiles"] = [None] * JC
            emit_sim(c, 0)
            emit_exp(c, 0)
            return c

        def emit_sim(c, j):
            hp = c["hp"]
            stt = psim.tile([P, 1024], FP, tag="sim", name="st_t")
            c["jtiles"][j] = stt
            for half in range(2):
                nc.tensor.matmul(
                    stt[:, half * 512:(half + 1) * 512],
                    lhsT=k_s[half * 64:(half + 1) * 64,
                             hp * M + j * P: hp * M + (j + 1) * P],
                    rhs=(c["qA"] if half == 0 else c["qB"]),
                )
            if c["v_inline"]:
                emit_v(j)

        def emit_exp(c, j):
            stt = c["jtiles"][j]
            in_v = stt.rearrange("p (hh n) -> p hh n", hh=2)
            out_v = p_v[:, c["par"], :, j, :]
            if j in DVE_JS:
                nc.vector.tensor_scalar(
                    out=out_v.bitcast(I8), in0=in_v,
                    scalar1=A8, scalar2=B8,
                    op0=mybir.AluOpType.mult, op1=mybir.AluOpType.add)
            elif j in POOL_JS:
                nc.gpsimd.tensor_scalar(
                    out=out_v.bitcast(I8), in0=in_v,
                    scalar1=A8, scalar2=B8,
                    op0=mybir.AluOpType.mult, op1=mybir.AluOpType.add)
            else:
                nc.scalar.activation(out=out_v, in_=in_v, func=EXP, scale=SIMSCALE)

        def emit_pv(c, g):
            hp, par = c["hp"], c["par"]
            for half in range(2):
                h = 2 * hp + half
                nc.tensor.matmul(
                    c["pv"][0:65, half * 512:(half + 1) * 512],
                    lhsT=vaug4[:, 2 * g:2 * g + 2, h, 0:65],
                    rhs=p_v[:, par, half, 2 * g:2 * g + 2, :],
                    perf_mode=DR,
                    start=(g == 0), stop=(g == GC - 1),
                )

        def pass_main(c, mine, last=False):
            # sims run 4 j-chunks ahead of pv so the previous pass's norm
            # chain (den->rec->broadcast->muls) never blocks pv(0) on the
            # single pv psum buffer.  The last pass has no successor, so it
            # runs pv with no lag to finish (and start the final norm+y
            # drain) as early as possible.
            lag = 0 if last else 2
            if last:
                emit_pv(c, 0)
                emit_pv(c, 1)
            for g in range(GC - 2):
                emit_sim(c, 2 * g + 4)
                emit_exp(c, 2 * g + 4)
                emit_sim(c, 2 * g + 5)
                emit_exp(c, 2 * g + 5)
                emit_pv(c, g + 2 - lag)
                if mine:
                    mine.popleft()()
                elif free:
                    free.popleft()()

        def pass_finish(c):
            """pv(7) + normalization (emitted after the NEXT pass's first
            sim+exp).  Denominators sit at psum partition 64; reciprocal ->
            bf16, K=1 ones-matmul broadcasts 1/den across partitions into
            psum, two muls write the normalized [64, 512] out tiles."""
            if not c.get("last"):
                emit_pv(c, GC - 2)
                emit_pv(c, GC - 1)
            pv, hp, it = c["pv"], c["hp"], c["it"]
            den = spool.tile([1, 1024], FP, tag="den", bufs=2)
            nc.scalar.copy(out=den, in_=pv[64:65, :])
            rec = spool.tile([1, 1024], FP, tag="rec", bufs=2)
            nc.vector.reciprocal_approx_fast(out=rec, in_=den)
            if dbg is not None and hp == 0 and it == 0:
                pvd = spool.tile([65, 1024], FP, tag="pvd", bufs=1)
                nc.vector.tensor_copy(out=pvd, in_=pv)
                nc.sync.dma_start(out=dbg["dbg_pv"][:, :], in_=pvd)
                nc.sync.dma_start(out=dbg["dbg_rec"][:, :], in_=rec)
                nc.sync.dma_start(out=dbg["dbg_den"][:, :], in_=den)
            bc = spool.tile([P, 1024], FP, tag="bc", bufs=2)
            nc.gpsimd.partition_broadcast(bc, rec[0:1, :])
            if dbg is not None and hp == 0 and it == 0:
                nc.sync.dma_start(out=dbg["dbg_bc"][:, :], in_=bc)
            ocol = hp * N + it * 512
            nc.vector.tensor_mul(
                out=out_s[0:64, ocol:ocol + 512],
                in0=pv[0:64, 0:512], in1=bc[0:64, 0:512])
            bb = spool.tile([D, 512], BF16, tag="bb", bufs=2)
            nc.vector.tensor_mul(
                out=bb, in0=pv[0:64, 512:1024], in1=bc[0:64, 512:1024])
            nc.sync.dma_start(out=out_s[64:128, ocol:ocol + 512], in_=bb)
            if hp == 1:
                for oc in range(CC):
                    free.append(lambda oc=oc, it=it: emit_y(oc, it))

        prev = None
        for hp in range(HPL):
            for it in range(IT):
                c = pass_begin(hp, it, emit_v_inline=(hp == 0 and it == 0))
                emit_sim(c, 1)
                emit_exp(c, 1)
                if prev is not None:
                    pass_finish(prev)
                emit_sim(c, 2)
                emit_exp(c, 2)
                emit_sim(c, 3)
                emit_exp(c, 3)
                if hp == 0 and it == 0:
                    emit_k(0, 1)
                c["last"] = (hp == HPL - 1 and it == IT - 1)
                pass_main(c, pinned[hp * IT + it], last=c["last"])
                prev = c
        pass_finish(prev)
        while free:
            free.popleft()()
        if dbg is not None:
            nc.sync.dma_start(out=dbg["dbg_q"][:, :], in_=q_s)
            nc.sync.dma_start(out=dbg["dbg_k"][:, :], in_=k_s)
            nc.sync.dma_start(out=dbg["dbg_vaug"][:, :], in_=vaug)
            nc.sync.dma_start(out=dbg["dbg_p"][:, :], in_=p_s)
            nc.sync.dma_start(out=dbg["dbg_out4"][:, :], in_=out_s)


# ------------------------- host-side shard / gather -------------------------

def _bf16(a):
    import ml_dtypes
    return np.ascontiguousarray(a.astype(ml_dtypes.bfloat16))


def _f8(a):
    import ml_dtypes
    return np.ascontiguousarray(a.astype(ml_dtypes.float8_e4m3))


def _shard_inputs(x, context, Wq, Wk, Wv, Wo, bo):
    """Build the per-core DRAM images (all [partitions, free])."""
    def chunk_rows(a):
        n = a.shape[1]
        return np.ascontiguousarray(
            a.reshape(-1, P, n).transpose(1, 0, 2).reshape(P, -1))

    WqT, WkT, WvT, WoT = Wq.T, Wk.T, Wv.T, Wo.T

    in_maps = []
    for c in range(N_CORES):
        b, hg = c // 2, c % 2
        cols = slice(hg * CIN, (hg + 1) * CIN)
        x_s = x[b].reshape(CC, P, IT, 512).transpose(1, 2, 0, 3).reshape(P, IT * CC * 512)
        ctx_s = context[b].reshape(CC, P, NT, 512).transpose(1, 2, 0, 3).reshape(P, NT * CC * 512)
        in_maps.append({
            "x": _f8(x_s),
            "ctx": _f8(ctx_s),
            "wq": _f8(chunk_rows(np.ascontiguousarray(WqT[:, cols])) * WSCALE),
            "wk": _f8(chunk_rows(np.ascontiguousarray(WkT[:, cols])) * WSCALE),
            "wv": _f8(chunk_rows(np.ascontiguousarray(WvT[:, cols])) * WSCALE),
            "wo": _bf16(chunk_rows(np.ascontiguousarray(WoT[hg * CIN:(hg + 1) * CIN, :]))),
        })
    return in_maps


def _gather_outputs(results, bo):
    y_full = np.empty((4, C, N), np.float32)
    for b in range(4):
        acc = None
        for hg in range(2):
            y_s = np.asarray(results[2 * b + hg]["y"], np.float32)  # [128, 4*2048]
            part = y_s.reshape(P, CC, N).transpose(1, 0, 2).reshape(C, N)
            acc = part if acc is None else acc + part
        y_full[b] = acc + bo[:, None]
    return y_full


_PROGRAM = None


def _get_program():
    global _PROGRAM
    if _PROGRAM is None:
        _PROGRAM = _build_program()
    return _PROGRAM


def run(trace=False, **inputs):
    nc = _get_program()
    bo = np.asarray(inputs["bo"], np.float32)
    in_maps = _shard_inputs(
        np.asarray(inputs["x"], np.float32),
        np.asarray(inputs["context"], np.float32),
        np.asarray(inputs["Wq"], np.float32),
        np.asarray(inputs["Wk"], np.float32),
        np.asarray(inputs["Wv"], np.float32),
        np.asarray(inputs["Wo"], np.float32),
        bo,
    )
    res = run_bass_kernel_spmd(nc, in_maps, list(range(N_CORES)), trace=trace)
    return _gather_outputs(res.results, bo), res


def kernel(**inputs):
    out, _ = run(trace=False, **inputs)
    return out
